# revision 12
# baseline (speedup 1.0000x reference)
"""CDMamba Trainium2 kernel.

Sharding: 8 cores = 4 batches x 2 halves of d_inner (512 channels each).
Each core computes the full x-path (in_proj x-part, conv, silu, x_proj) for
all 1024 channels (duplicated across the pair, so the d_inner contraction in
x_proj needs no collective); the z-path, dt, selective scan, gating and
out_proj run on the local 512 channels only.  out_proj partials (contraction
over d_inner) are summed on the host.  The per-core channel order is
permuted on the host so the local half is always channels 0..511 — the
program is SPMD-identical, only the data differs.

On-chip layout is d-major: [d partitions, time free].  The selective scan
uses the DVE tensor_tensor_scan instruction (state = a_t*state + u_t per
partition row), one scan per (d-tile, state index n).  The n-contraction
y = sum_n C_n * h_n accumulates in PSUM via identity matmuls on the tensor
engine; the D*x skip term is folded in as one extra diagonal matmul.

The DVE (vector engine) is the bottleneck: the scan runs at 2 cycles per
free element and can ONLY run there (gpsimd shares its SBUF port with the
DVE, so offloading the elementwise muls to gpsimd slows the scans down and
loses).  Therefore the schedule maximises DVE occupancy: phases are emitted
in the order A(f) A(r) B(f) A(g) B(r) B(g)+C so the tensor/scalar-bound
phase A of branches r and g overlaps the DVE-bound phase B of earlier
branches (per-engine queues are in emission order; A(x) is emitted *before*
the B slice it overlaps so its tensor work is not stuck behind B's
PSUM-accumulation matmuls).  To make the working sets coexist in SBUF,
dt round-trips through DRAM and the B/C broadcast tiles are loaded n-outer
with rotating buffers (the 2x4 dti split keeps PSUM at exactly 8 banks).
Softplus Exp/Ln activations are batched per phase to avoid reloading the
scalar-engine activation table (1.28us per function switch).
"""

import sys

import numpy as np

for _p in ("/opt/trn_rl_repo",):
    if _p not in sys.path:
        sys.path.insert(0, _p)

import concourse.bass as bass
import concourse.bacc as bacc
import concourse.tile as tile
from concourse import library_config
from concourse import mybir
from concourse.bass_utils import run_bass_kernel_spmd

F16 = mybir.dt.float16
F32 = mybir.dt.float32
AF = mybir.ActivationFunctionType
OP = mybir.AluOpType

D_MODEL = 512
D_STATE = 16
D_CONV = 4
D_INNER = 1024
DT_RANK = 32
NB = 4
NCORE = 8
DLOC = 512
L_FULL = 4096

BRANCHES = ("f", "r", "g")
TBI = {"f": 0, "r": 1, "g": 2}


def rev_ap(src, n):
    """Reversed view (along free dim) of the first n columns ending at src's
    last column."""
    return bass.AP(tensor=src.tensor, offset=src.offset + (n - 1),
                   ap=[list(src.ap[0]), [-1, n]])


def build(L, TA=512, TB=1024):
    """Build the SPMD Bass program for sequence length L."""
    TA = min(TA, L)
    TB = min(TB, L)
    assert L % TA == 0 and L % TB == 0
    nlt_a = L // TA
    nlt_b = L // TB
    nqb = TB // 512

    nc = bacc.Bacc()

    # ---- I/O ----
    inp = {t: nc.declare_dram_parameter(f"inp_{t}", [D_MODEL, L], F16, isOutput=False)
           for t in ("f", "g")}
    Wx = nc.declare_dram_parameter("Wx", [D_MODEL, D_INNER], F16, isOutput=False)
    Wz = nc.declare_dram_parameter("Wz", [D_MODEL, DLOC], F16, isOutput=False)
    Gx = nc.declare_dram_parameter("Gx", [D_MODEL, D_INNER], F16, isOutput=False)
    Gz = nc.declare_dram_parameter("Gz", [D_MODEL, DLOC], F16, isOutput=False)
    cwdiag = nc.declare_dram_parameter("cwdiag", [128, 96 * 128], F16, isOutput=False)
    cb = {t: nc.declare_dram_parameter(f"cb_{t}", [128, 8], F32, isOutput=False) for t in BRANCHES}
    xp = {t: nc.declare_dram_parameter(f"xp_{t}", [D_INNER, 64], F16, isOutput=False) for t in BRANCHES}
    dtw = {t: nc.declare_dram_parameter(f"dtw_{t}", [DT_RANK, DLOC], F32, isOutput=False) for t in BRANCHES}
    dtb = {t: nc.declare_dram_parameter(f"dtb_{t}", [128, 4], F32, isOutput=False) for t in BRANCHES}
    An = {t: nc.declare_dram_parameter(f"An_{t}", [128, 64], F32, isOutput=False) for t in BRANCHES}
    ddiag = nc.declare_dram_parameter("ddiag", [128, 12 * 128], F16, isOutput=False)
    opw = nc.declare_dram_parameter("opw", [DLOC, D_MODEL], F16, isOutput=False)
    ident = nc.declare_dram_parameter("ident", [128, 128], F16, isOutput=False)
    outp = nc.declare_dram_parameter("outp", [D_MODEL, L], F32, isOutput=True)

    # ---- DRAM scratch ----
    xz_d = nc.dram_tensor("xz_d", [D_INNER, L], F16)
    xs_d = {t: nc.dram_tensor(f"xs_{t}", [DLOC, L], F16) for t in BRANCHES}
    szl_d = {t: nc.dram_tensor(f"szl_{t}", [DLOC, L], F16) for t in ("f", "g")}
    y_d = {t: nc.dram_tensor(f"y_{t}", [DLOC, L], F16) for t in BRANCHES}
    ddt_d = {t: nc.dram_tensor(f"ddt_{t}", [DT_RANK, L], F32) for t in BRANCHES}
    bc_d = {t: nc.dram_tensor(f"bc_{t}", [32, L], F16) for t in BRANCHES}
    dt_d = {t: nc.dram_tensor(f"dt_{t}", [DLOC, L], F16) for t in BRANCHES}

    with tile.TileContext(nc) as tc:
        with tc.tile_pool(name="wpool", bufs=1) as wp, \
             tc.tile_pool(name="pa_ps", bufs=3, space="PSUM") as pa_ps, \
             tc.tile_pool(name="pa_sm", bufs=1, space="PSUM") as pa_sm, \
             tc.tile_pool(name="pb", bufs=2) as pb, \
             tc.tile_pool(name="pb_yps", bufs=1, space="PSUM") as pb_yps, \
             tc.tile_pool(name="paw", bufs=1) as paw, \
             tc.tile_pool(name="pa", bufs=2) as pa, \
             tc.tile_pool(name="pbc", bufs=1) as pbc, \
             tc.tile_pool(name="pc", bufs=2) as pc:

            # ---- persistent weights in SBUF ----
            def load_w(pool, name, dram, parts, width, dtype=F16):
                tl = []
                for k in range(parts):
                    t_ = pool.tile([128, width], dtype, tag=f"{name}{k}", name=f"{name}{k}")
                    nc.sync.dma_start(t_[:], dram[k * 128:(k + 1) * 128, :])
                    tl.append(t_)
                return tl

            xp_sb, dtw_sb, cb_sb, dtb_sb, An_sb = {}, {}, {}, {}, {}
            for t in BRANCHES:
                xp_sb[t] = wp.tile([128, 8 * 64], F16, tag=f"xp_{t}", name=f"xp_{t}")
                for k in range(8):
                    nc.sync.dma_start(xp_sb[t][:, k * 64:(k + 1) * 64],
                                      xp[t][k * 128:(k + 1) * 128, :])
                dtw_sb[t] = wp.tile([DT_RANK, DLOC], F32, tag=f"dtw_{t}", name=f"dtw_{t}")
                nc.sync.dma_start(dtw_sb[t][:], dtw[t][:])
                for nm, src, shape in (("cb", cb, [128, 8]),
                                       ("dtb", dtb, [128, 4]), ("An", An, [128, 64])):
                    t_ = wp.tile(shape, F32, tag=f"{nm}_{t}")
                    nc.sync.dma_start(t_[:], src[t][:])
                    {"cb": cb_sb, "dtb": dtb_sb, "An": An_sb}[nm][t] = t_
            op_sb = load_w(wp, "op", opw, 4, D_MODEL)
            id_sb = wp.tile([128, 128], F16, tag="ident", name="ident")
            nc.sync.dma_start(id_sb[:], ident[:])
            dd_sb = wp.tile([128, 12 * 128], F16, tag="ddg", name="ddg")
            nc.sync.dma_start(dd_sb[:], ddiag[:])
            zt_sb = wp.tile([128, 4], F16, tag="zt4", name="zt4")
            nc.gpsimd.memset(zt_sb[:], 0.0)

            wx_sb = {"Wx": load_w(paw, "Wx", Wx, 4, D_INNER),
                     "Gx": load_w(paw, "Gx", Gx, 4, D_INNER)}
            wz_sb = {"Wz": load_w(paw, "Wz", Wz, 4, DLOC),
                     "Gz": load_w(paw, "Gz", Gz, 4, DLOC)}
            in_x_sb = {"f": wx_sb["Wx"], "g": wx_sb["Gx"]}
            in_z_sb = {"f": wz_sb["Wz"], "g": wz_sb["Gz"]}

            tc.strict_bb_all_engine_barrier()

            # ========== dt prep: dt = softplus(dtw @ ddt + dtb) -> dt_d ====
            def dtprep_lt(t, lt):
                t0 = lt * TB
                ddt_sb = pb.tile([DT_RANK, TB], F32, tag="ddt_sb", name="ddt_sb",
                                 bufs=1)
                nc.gpsimd.dma_start(ddt_sb[:], ddt_d[t][:, t0:t0 + TB])
                # softplus(x) = ln(exp(x) + 1); x in [-8, 2] here.  All Exp
                # activations run back-to-back, then all Ln — alternating
                # them reloads the scalar act table (1.28us per switch).
                rows = []
                spes = []
                for dti in range(4):
                    dtt = pb.tile([128, TB], F16, tag=f"dtp{dti}", name=f"dtp{dti}", bufs=1)
                    for q in range(nqb):
                        dps = pa_ps.tile([128, 512], F32, tag="ps", name="ps")
                        nc.tensor.matmul(dps[:],
                                         dtw_sb[t][:, dti * 128:(dti + 1) * 128],
                                         ddt_sb[:, q * 512:(q + 1) * 512],
                                         start=True, stop=True)
                        spe = pb.tile([128, 512], F16, tag=f"spe{dti}_{q}",
                                      name=f"spe{dti}_{q}", bufs=1)
                        nc.scalar.activation(spe[:], dps[:], AF.Exp,
                                             bias=dtb_sb[t][:, dti:dti + 1])
                        spes.append((dtt, q, spe))
                    rows.append(dtt)
                for dtt, q, spe in spes:
                    nc.scalar.activation(dtt[:, q * 512:(q + 1) * 512], spe[:],
                                         AF.Ln, bias=1.0)
                for dti in range(4):
                    nc.sync.dma_start(dt_d[t][dti * 128:(dti + 1) * 128, t0:t0 + TB],
                                      rows[dti][:])

            # ========== PHASE A: in_proj + conv(PE) + silu + x_proj ==========
            def phaseA(t):
                tb = TBI[t]
                cwd_sb = paw.tile([128, 32 * 128], F16, tag="cwd", name="cwd",
                                  bufs=1)
                nc.sync.dma_start(cwd_sb[:],
                                  cwdiag[:, tb * 32 * 128:(tb + 1) * 32 * 128])
                xz_prev = [None] * 8
                for lt in range(nlt_a):
                    t0 = lt * TA
                    rhs = []
                    if t != "r":
                        for k in range(4):
                            r_ = pa.tile([128, TA], F16, tag=f"rhs{k}", name=f"rhs{k}", bufs=1)
                            nc.sync.dma_start(r_[:], inp[t][k * 128:(k + 1) * 128, t0:t0 + TA])
                            rhs.append(r_)
                    dbl_ps = pa_sm.tile([64, TA], F32, tag="dbl", name="dbl")
                    for d8 in range(8):
                        xz = pa.tile([128, TA + 3], F16, tag=f"xz{d8}", name=f"xz{d8}")
                        if t == "r":
                            # xz_r = time-reversed xz_f: forward DMA, then a
                            # reversed on-chip copy (negative-stride DRAM DMA
                            # lowers to per-element descriptors - never do it)
                            w = TA if lt == 0 else TA + 3
                            xzf = pa.tile([128, TA + 3], F16, tag="xzf", name="xzf", bufs=1)
                            nc.sync.dma_start(
                                xzf[:, 0:w],
                                xz_d[d8 * 128:(d8 + 1) * 128, L - t0 - TA:L - t0 - TA + w])
                            if lt == 0:
                                nc.gpsimd.tensor_copy(xz[:, 0:3], zt_sb[:, 0:3])
                                nc.gpsimd.tensor_copy(xz[:, 3:TA + 3], rev_ap(xzf[:, 0:TA], TA))
                            else:
                                nc.gpsimd.tensor_copy(xz[:], rev_ap(xzf[:, 0:TA + 3], TA + 3))
                        else:
                            ps = pa_ps.tile([128, TA], F32, tag="ps", name="ps")
                            for k in range(4):
                                nc.tensor.matmul(ps[:], in_x_sb[t][k][:, d8 * 128:(d8 + 1) * 128],
                                                 rhs[k][:], start=(k == 0), stop=(k == 3))
                            if lt == 0:
                                nc.scalar.copy(xz[:, 0:3], zt_sb[:, 0:3])
                            else:
                                nc.scalar.copy(xz[:, 0:3], xz_prev[d8][:, TA:TA + 3])
                            nc.scalar.copy(xz[:, 3:TA + 3], ps[:])
                            xz_prev[d8] = xz
                            if t == "f":
                                nc.sync.dma_start(xz_d[d8 * 128:(d8 + 1) * 128, t0:t0 + TA],
                                                  xz[:, 3:TA + 3])
                        # conv: 4 shifted diagonal matmuls accumulating in PSUM
                        cps = pa_ps.tile([128, TA], F32, tag="ps", name="ps")
                        for k in range(4):
                            w0 = (d8 * 4 + k) * 128
                            nc.tensor.matmul(cps[:], cwd_sb[:, w0:w0 + 128],
                                             xz[:, k:k + TA], start=(k == 0), stop=(k == 3))
                        xs = pa.tile([128, TA], F16, tag="xs", name="xs")
                        nc.scalar.activation(xs[:], cps[:], AF.Silu,
                                             bias=cb_sb[t][:, d8:d8 + 1])
                        nc.tensor.matmul(dbl_ps[:], xp_sb[t][:, d8 * 64:(d8 + 1) * 64],
                                         xs[:], start=(d8 == 0), stop=(d8 == 7))
                        if d8 < 4:  # local half (host permutes channels)
                            nc.sync.dma_start(xs_d[t][d8 * 128:(d8 + 1) * 128, t0:t0 + TA],
                                              xs[:])
                    if t != "r":
                        for zt in range(4):
                            zps = pa_ps.tile([128, TA], F32, tag="ps", name="ps")
                            for k in range(4):
                                nc.tensor.matmul(zps[:], in_z_sb[t][k][:, zt * 128:(zt + 1) * 128],
                                                 rhs[k][:], start=(k == 0), stop=(k == 3))
                            sz = pa.tile([128, TA], F16, tag="sz", name="sz")
                            nc.scalar.activation(sz[:], zps[:], AF.Silu)
                            nc.sync.dma_start(szl_d[t][zt * 128:(zt + 1) * 128, t0:t0 + TA], sz[:])
                    ddt_t = pa.tile([DT_RANK, TA], F32, tag="ddt", name="ddt", bufs=1)
                    nc.scalar.copy(ddt_t[:], dbl_ps[0:DT_RANK, :])
                    nc.sync.dma_start(ddt_d[t][:, t0:t0 + TA], ddt_t[:])
                    bc_t = pa.tile([32, TA], F16, tag="bct", name="bct", bufs=1)
                    nc.scalar.copy(bc_t[:], dbl_ps[DT_RANK:64, :])
                    nc.sync.dma_start(bc_d[t][:, t0:t0 + TA], bc_t[:])
                    if (lt + 1) * TA % TB == 0:
                        dtprep_lt(t, ((lt + 1) * TA) // TB - 1)

            # ========== PHASE B: selective scan ==========
            def phaseB(t, after_lt=None):
                tb = TBI[t]
                carries = [None] * 4
                for lt in range(nlt_b):
                    t0 = lt * TB
                    # per-lt inputs shared across n: dt, xs, dxt = dt*xs
                    dtt_t, xst_t, dxt_t = [], [], []
                    for dti in range(4):
                        dtt = pb.tile([128, TB], F16, tag=f"dtt{dti}", name=f"dtt{dti}", bufs=1)
                        nc.gpsimd.dma_start(dtt[:], dt_d[t][dti * 128:(dti + 1) * 128, t0:t0 + TB])
                        xst = pb.tile([128, TB], F16, tag=f"xs{dti}", name=f"xs{dti}", bufs=1)
                        nc.gpsimd.dma_start(xst[:], xs_d[t][dti * 128:(dti + 1) * 128, t0:t0 + TB])
                        dxt = pb.tile([128, TB], F16, tag=f"dtx{dti}", name=f"dtx{dti}", bufs=1)
                        nc.vector.tensor_mul(dxt[:], dtt[:], xst[:])
                        dtt_t.append(dtt)
                        xst_t.append(xst)
                        dxt_t.append(dxt)
                    cnew = [pb.tile([128, D_STATE], F32, tag=f"carry{dti}",
                                    name=f"carry{dti}") for dti in range(4)]
                    for half in range(2):
                        dts = (2 * half, 2 * half + 1)
                        yps = {dti: pb_yps.tile([128, TB], F32, tag=f"yps{i}",
                                                name=f"yps{i}")
                               for i, dti in enumerate(dts)}
                        for n in range(D_STATE):
                            bn = pbc.tile([128, TB], F16, tag="Bn", name="Bn", bufs=3)
                            nc.gpsimd.dma_start(
                                bn[:], bc_d[t][n:n + 1, t0:t0 + TB].partition_broadcast(128))
                            cn = pbc.tile([128, TB], F16, tag="Cn", name="Cn", bufs=3)
                            nc.gpsimd.dma_start(
                                cn[:], bc_d[t][16 + n:17 + n, t0:t0 + TB].partition_broadcast(128))
                            for i, dti in enumerate(dts):
                                a_t = pb.tile([128, TB], F16, tag=f"a{i}", name=f"a{i}", bufs=3)
                                nc.scalar.activation(a_t[:], dtt_t[dti][:], AF.Exp,
                                                     scale=An_sb[t][:, dti * 16 + n:dti * 16 + n + 1])
                                u_t = pb.tile([128, TB], F16, tag=f"u{i}", name=f"u{i}")
                                nc.vector.tensor_mul(u_t[:], dxt_t[dti][:], bn[:])
                                h_t = pb.tile([128, TB], F16, tag=f"h{i}", name=f"h{i}")
                                init = 0.0 if lt == 0 else carries[dti][:, n:n + 1]
                                nc.vector.tensor_tensor_scan(h_t[:], a_t[:], u_t[:], init,
                                                             op0=OP.mult, op1=OP.add)
                                if lt < nlt_b - 1:
                                    nc.gpsimd.tensor_copy(cnew[dti][:, n:n + 1],
                                                          h_t[:, TB - 1:TB])
                                tmp = pb.tile([128, TB], F16, tag=f"tmp{i}", name=f"tmp{i}", bufs=4)
                                nc.vector.tensor_mul(tmp[:], h_t[:], cn[:])
                                for q in range(nqb):
                                    nc.tensor.matmul(yps[dti][:, q * 512:(q + 1) * 512], id_sb[:],
                                                     tmp[:, q * 512:(q + 1) * 512],
                                                     start=(n == 0), stop=False)
                        for dti in dts:
                            # D*x skip term as one diagonal matmul, closes the group
                            w0 = (tb * 4 + dti) * 128
                            for q in range(nqb):
                                nc.tensor.matmul(yps[dti][:, q * 512:(q + 1) * 512],
                                                 dd_sb[:, w0:w0 + 128],
                                                 xst_t[dti][:, q * 512:(q + 1) * 512],
                                                 start=False, stop=True)
                            szt = pb.tile([128, TB], F16, tag="szt", name="szt", bufs=1)
                            if t == "r":
                                szf = pb.tile([128, TB], F16, tag="szf", name="szf", bufs=1)
                                nc.gpsimd.dma_start(
                                    szf[:], szl_d["f"][dti * 128:(dti + 1) * 128,
                                                       L - t0 - TB:L - t0])
                                nc.gpsimd.tensor_copy(szt[:], rev_ap(szf[:], TB))
                            else:
                                nc.gpsimd.dma_start(szt[:],
                                                    szl_d[t][dti * 128:(dti + 1) * 128, t0:t0 + TB])
                            ysb = pb.tile([128, TB], F16, tag="ysb", name="ysb", bufs=1)
                            nc.scalar.copy(ysb[:], yps[dti][:])
                            yo = pb.tile([128, TB], F16, tag="yo", name="yo")
                            nc.vector.tensor_mul(yo[:], ysb[:], szt[:])
                            nc.gpsimd.dma_start(y_d[t][dti * 128:(dti + 1) * 128, t0:t0 + TB],
                                                yo[:])
                    carries = cnew
                    if after_lt is not None:
                        after_lt(lt)

            # ========== PHASE C: combine + out_proj ==========
            def phaseC_lt(lt):
                t0 = lt * TB
                Y_t = []
                for dti in range(4):
                    yf = pc.tile([128, TB], F16, tag="yf", name="yf", bufs=1)
                    nc.gpsimd.dma_start(yf[:], y_d["f"][dti * 128:(dti + 1) * 128, t0:t0 + TB])
                    yg = pc.tile([128, TB], F16, tag="yg", name="yg", bufs=1)
                    nc.gpsimd.dma_start(yg[:], y_d["g"][dti * 128:(dti + 1) * 128, t0:t0 + TB])
                    yrf = pc.tile([128, TB], F16, tag="yrf", name="yrf", bufs=1)
                    nc.gpsimd.dma_start(
                        yrf[:], y_d["r"][dti * 128:(dti + 1) * 128, L - t0 - TB:L - t0])
                    yr = pc.tile([128, TB], F16, tag=f"Y{dti}", name=f"Y{dti}", bufs=1)
                    nc.gpsimd.tensor_copy(yr[:], rev_ap(yrf[:], TB))
                    sg = pc.tile([128, TB], F16, tag="sg", name="sg", bufs=1)
                    nc.scalar.activation(sg[:], yg[:], AF.Silu)
                    nc.gpsimd.tensor_add(yr[:], yf[:], yr[:])
                    nc.gpsimd.tensor_mul(yr[:], yr[:], sg[:])
                    Y_t.append(yr)
                for mt in range(4):
                    for q in range(nqb):
                        ops = pa_ps.tile([128, 512], F32, tag="ps", name="ps")
                        for dti in range(4):
                            nc.tensor.matmul(ops[:], op_sb[dti][:, mt * 128:(mt + 1) * 128],
                                             Y_t[dti][:, q * 512:(q + 1) * 512],
                                             start=(dti == 0), stop=(dti == 3))
                        ot = pc.tile([128, 512], F32, tag="ot", name="ot", bufs=1)
                        nc.scalar.copy(ot[:], ops[:])
                        nc.sync.dma_start(
                            outp[mt * 128:(mt + 1) * 128, t0 + q * 512:t0 + (q + 1) * 512],
                            ot[:])

            # ---- schedule: overlap A(r)/A(g) under B(f)/B(r) ----
            phaseA("f")
            phaseA("r")
            phaseB("f")
            phaseA("g")
            phaseB("r")
            phaseB("g", after_lt=phaseC_lt)

    nc.finalize()
    return nc


def prep_core_inputs(inputs, c, L):
    """Build the input dict for core c (b = c//2, dh = c%2).

    Channels of d_inner are permuted per core so the local half is always
    first: perm = [dh*512 .. dh*512+511, other half].
    """
    b, dh = divmod(c, 2)
    f16 = np.float16
    f32 = np.float32
    perm = np.concatenate([np.arange(dh * 512, dh * 512 + 512),
                           np.arange((1 - dh) * 512, (1 - dh) * 512 + 512)])
    loc = perm[:512]

    hid = np.asarray(inputs["hidden_states"][b], dtype=f32)[:L]
    ano = np.asarray(inputs["another_hidden_states"][b], dtype=f32)[:L]
    d = {
        "inp_f": np.ascontiguousarray(hid.T).astype(f16),
        "inp_g": np.ascontiguousarray(ano.T).astype(f16),
        "Wx": np.ascontiguousarray(inputs["in_proj_w"][:D_INNER][perm].T).astype(f16),
        "Wz": np.ascontiguousarray(inputs["in_proj_w"][D_INNER:][loc].T).astype(f16),
        "Gx": np.ascontiguousarray(inputs["in_proj_g_w"][:D_INNER][perm].T).astype(f16),
        "Gz": np.ascontiguousarray(inputs["in_proj_g_w"][D_INNER:][loc].T).astype(f16),
        "opw": np.ascontiguousarray(inputs["out_proj_w"][:, loc].T).astype(f16),
        "ident": np.eye(128, dtype=f16),
    }
    cwdiag = np.zeros((128, 96 * 128), dtype=f16)
    ddiag = np.zeros((128, 12 * 128), dtype=f16)
    di = np.arange(128)
    for tb, (t, cwn, cbn, xpn, dtwn, dtbn, alogn, dn) in enumerate((
            ("f", "convw_f", "convb_f", "xproj_f", "dtw_f", "dtb_f", "Alog_f", "D_f"),
            ("r", "convw_r", "convb_r", "xproj_r", "dtw_r", "dtb_r", "Alog_r", "D_r"),
            ("g", "convw_g", "convb_g", "xproj_g", "dtw_g", "dtb_g", "Alog_g", "D_g"))):
        cwp = np.asarray(inputs[cwn], f32)[perm]          # (1024, 4)
        for d8 in range(8):
            for k in range(4):
                w0 = (tb * 32 + d8 * 4 + k) * 128
                cwdiag[di, w0 + di] = cwp[d8 * 128:(d8 + 1) * 128, k].astype(f16)
        cbp = np.asarray(inputs[cbn], f32)[perm]          # (1024,)
        d[f"cb_{t}"] = np.ascontiguousarray(cbp.reshape(8, 128).T).astype(f32)
        d[f"xp_{t}"] = np.ascontiguousarray(np.asarray(inputs[xpn], f32).T[perm]).astype(f16)
        d[f"dtw_{t}"] = np.ascontiguousarray(np.asarray(inputs[dtwn], f32)[loc].T).astype(f32)
        dtbp = np.asarray(inputs[dtbn], f32)[loc]
        d[f"dtb_{t}"] = np.ascontiguousarray(dtbp.reshape(4, 128).T).astype(f32)
        Afull = -np.exp(np.asarray(inputs[alogn], f32))[loc]   # (512, 16)
        d[f"An_{t}"] = np.ascontiguousarray(
            Afull.reshape(4, 128, 16).transpose(1, 0, 2).reshape(128, 64)).astype(f32)
        Dp = np.asarray(inputs[dn], f32)[loc]
        for dti in range(4):
            w0 = (tb * 4 + dti) * 128
            ddiag[di, w0 + di] = Dp[dti * 128:(dti + 1) * 128].astype(f16)
    d["cwdiag"] = cwdiag
    d["ddiag"] = ddiag
    return d


_NC_CACHE = {}
TRACE = False
LAST_RESULT = None
BUILD_KW = {}


def kernel(**inputs):
    global LAST_RESULT
    L = inputs["hidden_states"].shape[1]
    key = (L, tuple(sorted(BUILD_KW.items())))
    if key not in _NC_CACHE:
        _NC_CACHE[key] = build(L, **BUILD_KW)
    nc = _NC_CACHE[key]
    in_maps = [prep_core_inputs(inputs, c, L) for c in range(NCORE)]
    res = run_bass_kernel_spmd(nc, in_maps, core_ids=list(range(NCORE)),
                               trace=TRACE)
    LAST_RESULT = res
    outs = []
    for b in range(NB):
        p = res.results[2 * b]["outp"].astype(np.float32) + \
            res.results[2 * b + 1]["outp"].astype(np.float32)
        outs.append(p.T)
    return np.stack(outs).astype(np.float32)


if __name__ == "__main__":
    nc = build(512)
    print("built ok")


# revision 14
# speedup vs baseline: 1.0652x; 1.0652x over previous
"""CDMamba Trainium2 kernel.

Sharding: 8 cores = 4 batches x 2 halves of d_inner (512 channels each).
Each core computes the full x-path (in_proj x-part, conv, silu, x_proj) for
all 1024 channels (duplicated across the pair, so the d_inner contraction in
x_proj needs no collective); the z-path, dt, selective scan, gating and
out_proj run on the local 512 channels only.  out_proj partials (contraction
over d_inner) are summed on the host.  The per-core channel order is
permuted on the host so the local half is always channels 0..511 — the
program is SPMD-identical, only the data differs.

On-chip layout is d-major: [d partitions, time free].  The selective scan
uses the DVE tensor_tensor_scan instruction (state = a_t*state + u_t per
partition row), one scan per (d-tile, state index n).  The n-contraction
y = sum_n C_n * h_n accumulates in PSUM via identity matmuls on the tensor
engine; the D*x skip term is folded in as one extra diagonal matmul.

The DVE (vector engine) is the bottleneck: the scan runs at 2 cycles per
free element and can ONLY run there (gpsimd shares its SBUF port with the
DVE, so offloading the elementwise muls to gpsimd slows the scans down and
loses).  Therefore the schedule maximises DVE occupancy: phases are emitted
in the order A(f) A(r) B(f) A(g) B(r) B(g)+C so the tensor/scalar-bound
phase A of branches r and g overlaps the DVE-bound phase B of earlier
branches (per-engine queues are in emission order; A(x) is emitted *before*
the B slice it overlaps so its tensor work is not stuck behind B's
PSUM-accumulation matmuls).  To make the working sets coexist in SBUF,
dt round-trips through DRAM and the B/C broadcast tiles are loaded n-outer
with rotating buffers (the 2x4 dti split keeps PSUM at exactly 8 banks).
Softplus Exp/Ln activations are batched per phase to avoid reloading the
scalar-engine activation table (1.28us per function switch).
"""

import sys

import numpy as np

for _p in ("/opt/trn_rl_repo",):
    if _p not in sys.path:
        sys.path.insert(0, _p)

import concourse.bass as bass
import concourse.bacc as bacc
import concourse.tile as tile
from concourse import library_config
from concourse import mybir
from concourse.bass_utils import run_bass_kernel_spmd

F16 = mybir.dt.float16
F32 = mybir.dt.float32
AF = mybir.ActivationFunctionType
OP = mybir.AluOpType

D_MODEL = 512
D_STATE = 16
D_CONV = 4
D_INNER = 1024
DT_RANK = 32
NB = 4
NCORE = 8
DLOC = 512
L_FULL = 4096

BRANCHES = ("f", "r", "g")
TBI = {"f": 0, "r": 1, "g": 2}


def rev_ap(src, n):
    """Reversed view (along free dim) of the first n columns ending at src's
    last column."""
    return bass.AP(tensor=src.tensor, offset=src.offset + (n - 1),
                   ap=[list(src.ap[0]), [-1, n]])


def build(L, TA=512, TB=1024):
    """Build the SPMD Bass program for sequence length L."""
    TA = min(TA, L)
    TB = min(TB, L)
    assert L % TA == 0 and L % TB == 0
    nlt_a = L // TA
    nlt_b = L // TB
    nqb = TB // 512

    nc = bacc.Bacc()

    # ---- I/O ----
    inp = {t: nc.declare_dram_parameter(f"inp_{t}", [D_MODEL, L], F16, isOutput=False)
           for t in ("f", "g")}
    Wx = nc.declare_dram_parameter("Wx", [D_MODEL, D_INNER], F16, isOutput=False)
    Wz = nc.declare_dram_parameter("Wz", [D_MODEL, DLOC], F16, isOutput=False)
    Gx = nc.declare_dram_parameter("Gx", [D_MODEL, D_INNER], F16, isOutput=False)
    Gz = nc.declare_dram_parameter("Gz", [D_MODEL, DLOC], F16, isOutput=False)
    cwdiag = nc.declare_dram_parameter("cwdiag", [128, 96 * 128], F16, isOutput=False)
    cb = {t: nc.declare_dram_parameter(f"cb_{t}", [128, 8], F32, isOutput=False) for t in BRANCHES}
    xp = {t: nc.declare_dram_parameter(f"xp_{t}", [D_INNER, 64], F16, isOutput=False) for t in BRANCHES}
    dtw = {t: nc.declare_dram_parameter(f"dtw_{t}", [DT_RANK, DLOC], F32, isOutput=False) for t in BRANCHES}
    dtb = {t: nc.declare_dram_parameter(f"dtb_{t}", [128, 4], F32, isOutput=False) for t in BRANCHES}
    An = {t: nc.declare_dram_parameter(f"An_{t}", [128, 64], F32, isOutput=False) for t in BRANCHES}
    ddiag = nc.declare_dram_parameter("ddiag", [128, 12 * 128], F16, isOutput=False)
    opw = nc.declare_dram_parameter("opw", [DLOC, D_MODEL], F16, isOutput=False)
    ident = nc.declare_dram_parameter("ident", [128, 128], F16, isOutput=False)
    outp = nc.declare_dram_parameter("outp", [D_MODEL, L], F32, isOutput=True)

    # ---- DRAM scratch ----
    xz_d = nc.dram_tensor("xz_d", [D_INNER, L], F16)
    xs_d = {t: nc.dram_tensor(f"xs_{t}", [DLOC, L], F16) for t in BRANCHES}
    szl_d = {t: nc.dram_tensor(f"szl_{t}", [DLOC, L], F16) for t in ("f", "g")}
    y_d = {t: nc.dram_tensor(f"y_{t}", [DLOC, L], F16) for t in BRANCHES}
    ddt_d = {t: nc.dram_tensor(f"ddt_{t}", [DT_RANK, L], F32) for t in BRANCHES}
    bc_d = {t: nc.dram_tensor(f"bc_{t}", [32, L], F16) for t in BRANCHES}
    dt_d = {t: nc.dram_tensor(f"dt_{t}", [DLOC, L], F16) for t in BRANCHES}

    with tile.TileContext(nc) as tc:
        with tc.tile_pool(name="wpool", bufs=1) as wp, \
             tc.tile_pool(name="pa_ps", bufs=3, space="PSUM") as pa_ps, \
             tc.tile_pool(name="pa_sm", bufs=1, space="PSUM") as pa_sm, \
             tc.tile_pool(name="pb", bufs=2) as pb, \
             tc.tile_pool(name="pb_yps", bufs=1, space="PSUM") as pb_yps, \
             tc.tile_pool(name="paw", bufs=1) as paw, \
             tc.tile_pool(name="pa", bufs=2) as pa, \
             tc.tile_pool(name="pbc", bufs=1) as pbc, \
             tc.tile_pool(name="pc", bufs=2) as pc:

            # ---- persistent weights in SBUF ----
            def load_w(pool, name, dram, parts, width, dtype=F16):
                tl = []
                for k in range(parts):
                    t_ = pool.tile([128, width], dtype, tag=f"{name}{k}", name=f"{name}{k}")
                    nc.sync.dma_start(t_[:], dram[k * 128:(k + 1) * 128, :])
                    tl.append(t_)
                return tl

            xp_sb, dtw_sb, cb_sb, dtb_sb, An_sb = {}, {}, {}, {}, {}
            for t in BRANCHES:
                xp_sb[t] = wp.tile([128, 8 * 64], F16, tag=f"xp_{t}", name=f"xp_{t}")
                for k in range(8):
                    nc.sync.dma_start(xp_sb[t][:, k * 64:(k + 1) * 64],
                                      xp[t][k * 128:(k + 1) * 128, :])
                dtw_sb[t] = wp.tile([DT_RANK, DLOC], F32, tag=f"dtw_{t}", name=f"dtw_{t}")
                nc.sync.dma_start(dtw_sb[t][:], dtw[t][:])
                for nm, src, shape in (("cb", cb, [128, 8]),
                                       ("dtb", dtb, [128, 4]), ("An", An, [128, 64])):
                    t_ = wp.tile(shape, F32, tag=f"{nm}_{t}")
                    nc.sync.dma_start(t_[:], src[t][:])
                    {"cb": cb_sb, "dtb": dtb_sb, "An": An_sb}[nm][t] = t_
            op_sb = load_w(wp, "op", opw, 4, D_MODEL)
            id_sb = wp.tile([128, 128], F16, tag="ident", name="ident")
            nc.sync.dma_start(id_sb[:], ident[:])
            dd_sb = wp.tile([128, 12 * 128], F16, tag="ddg", name="ddg")
            nc.sync.dma_start(dd_sb[:], ddiag[:])
            zt_sb = wp.tile([128, 4], F16, tag="zt4", name="zt4")
            nc.gpsimd.memset(zt_sb[:], 0.0)

            wx_sb = {"Wx": load_w(paw, "Wx", Wx, 4, D_INNER),
                     "Gx": load_w(paw, "Gx", Gx, 4, D_INNER)}
            wz_sb = {"Wz": load_w(paw, "Wz", Wz, 4, DLOC),
                     "Gz": load_w(paw, "Gz", Gz, 4, DLOC)}
            in_x_sb = {"f": wx_sb["Wx"], "g": wx_sb["Gx"]}
            in_z_sb = {"f": wz_sb["Wz"], "g": wz_sb["Gz"]}

            tc.strict_bb_all_engine_barrier()

            # ========== dt prep: dt = softplus(dtw @ ddt + dtb) -> dt_d ====
            def dtprep_lt(t, lt):
                t0 = lt * TB
                ddt_sb = pb.tile([DT_RANK, TB], F32, tag="ddt_sb", name="ddt_sb",
                                 bufs=1)
                nc.gpsimd.dma_start(ddt_sb[:], ddt_d[t][:, t0:t0 + TB])
                # softplus(x) = ln(exp(x) + 1); x in [-8, 2] here.  All Exp
                # activations run back-to-back, then all Ln — alternating
                # them reloads the scalar act table (1.28us per switch).
                rows = []
                spes = []
                for dti in range(4):
                    dtt = pb.tile([128, TB], F16, tag=f"dtp{dti}", name=f"dtp{dti}", bufs=1)
                    for q in range(nqb):
                        dps = pa_ps.tile([128, 512], F32, tag="ps", name="ps")
                        nc.tensor.matmul(dps[:],
                                         dtw_sb[t][:, dti * 128:(dti + 1) * 128],
                                         ddt_sb[:, q * 512:(q + 1) * 512],
                                         start=True, stop=True)
                        spe = pb.tile([128, 512], F16, tag=f"spe{dti}_{q}",
                                      name=f"spe{dti}_{q}", bufs=1)
                        nc.scalar.activation(spe[:], dps[:], AF.Exp,
                                             bias=dtb_sb[t][:, dti:dti + 1])
                        spes.append((dtt, q, spe))
                    rows.append(dtt)
                for dtt, q, spe in spes:
                    nc.scalar.activation(dtt[:, q * 512:(q + 1) * 512], spe[:],
                                         AF.Ln, bias=1.0)
                for dti in range(4):
                    nc.sync.dma_start(dt_d[t][dti * 128:(dti + 1) * 128, t0:t0 + TB],
                                      rows[dti][:])

            # ========== PHASE A: in_proj + conv(PE) + silu + x_proj ==========
            def phaseA(t):
                tb = TBI[t]
                cwd_sb = paw.tile([128, 32 * 128], F16, tag="cwd", name="cwd",
                                  bufs=1)
                nc.sync.dma_start(cwd_sb[:],
                                  cwdiag[:, tb * 32 * 128:(tb + 1) * 32 * 128])
                xz_prev = [None] * 8
                for lt in range(nlt_a):
                    t0 = lt * TA
                    rhs = []
                    if t != "r":
                        for k in range(4):
                            r_ = pa.tile([128, TA], F16, tag=f"rhs{k}", name=f"rhs{k}", bufs=1)
                            nc.sync.dma_start(r_[:], inp[t][k * 128:(k + 1) * 128, t0:t0 + TA])
                            rhs.append(r_)
                    dbl_ps = pa_sm.tile([64, TA], F32, tag="dbl", name="dbl")
                    for d8 in range(8):
                        if t == "r":
                            # xz_r = time-reversed xz_f: forward DMA, then the
                            # conv matmuls read it through a negative-stride
                            # (reversed) moving AP — no on-chip reversal copy.
                            # Layout: xzf = forward data; window k of the
                            # reversed stream is reversed(xzf[3-k : 3-k+TA]).
                            # At lt==0 columns [TA, TA+3) are the zero history.
                            xzf = pa.tile([128, TA + 3], F16, tag="xzf", name="xzf", bufs=2)
                            if lt == 0:
                                nc.sync.dma_start(
                                    xzf[:, 0:TA],
                                    xz_d[d8 * 128:(d8 + 1) * 128, L - TA:L])
                                nc.gpsimd.memset(xzf[:, TA:TA + 3], 0.0)
                            else:
                                nc.sync.dma_start(
                                    xzf[:, 0:TA + 3],
                                    xz_d[d8 * 128:(d8 + 1) * 128, L - t0 - TA:L - t0 + 3])
                            conv_in = [rev_ap(xzf[:, 3 - k:3 - k + TA], TA)
                                       for k in range(4)]
                        else:
                            xz = pa.tile([128, TA + 3], F16, tag=f"xz{d8}", name=f"xz{d8}")
                            ps = pa_ps.tile([128, TA], F32, tag="ps", name="ps")
                            for k in range(4):
                                nc.tensor.matmul(ps[:], in_x_sb[t][k][:, d8 * 128:(d8 + 1) * 128],
                                                 rhs[k][:], start=(k == 0), stop=(k == 3))
                            if lt == 0:
                                nc.scalar.copy(xz[:, 0:3], zt_sb[:, 0:3])
                            else:
                                nc.scalar.copy(xz[:, 0:3], xz_prev[d8][:, TA:TA + 3])
                            nc.scalar.copy(xz[:, 3:TA + 3], ps[:])
                            xz_prev[d8] = xz
                            if t == "f":
                                nc.sync.dma_start(xz_d[d8 * 128:(d8 + 1) * 128, t0:t0 + TA],
                                                  xz[:, 3:TA + 3])
                            conv_in = [xz[:, k:k + TA] for k in range(4)]
                        # conv: 4 shifted diagonal matmuls accumulating in PSUM
                        cps = pa_ps.tile([128, TA], F32, tag="ps", name="ps")
                        for k in range(4):
                            w0 = (d8 * 4 + k) * 128
                            nc.tensor.matmul(cps[:], cwd_sb[:, w0:w0 + 128],
                                             conv_in[k], start=(k == 0), stop=(k == 3))
                        xs = pa.tile([128, TA], F16, tag="xs", name="xs")
                        nc.scalar.activation(xs[:], cps[:], AF.Silu,
                                             bias=cb_sb[t][:, d8:d8 + 1])
                        nc.tensor.matmul(dbl_ps[:], xp_sb[t][:, d8 * 64:(d8 + 1) * 64],
                                         xs[:], start=(d8 == 0), stop=(d8 == 7))
                        if d8 < 4:  # local half (host permutes channels)
                            nc.sync.dma_start(xs_d[t][d8 * 128:(d8 + 1) * 128, t0:t0 + TA],
                                              xs[:])
                    if t != "r":
                        for zt in range(4):
                            zps = pa_ps.tile([128, TA], F32, tag="ps", name="ps")
                            for k in range(4):
                                nc.tensor.matmul(zps[:], in_z_sb[t][k][:, zt * 128:(zt + 1) * 128],
                                                 rhs[k][:], start=(k == 0), stop=(k == 3))
                            sz = pa.tile([128, TA], F16, tag="sz", name="sz")
                            nc.scalar.activation(sz[:], zps[:], AF.Silu)
                            nc.sync.dma_start(szl_d[t][zt * 128:(zt + 1) * 128, t0:t0 + TA], sz[:])
                    ddt_t = pa.tile([DT_RANK, TA], F32, tag="ddt", name="ddt", bufs=1)
                    nc.scalar.copy(ddt_t[:], dbl_ps[0:DT_RANK, :])
                    nc.sync.dma_start(ddt_d[t][:, t0:t0 + TA], ddt_t[:])
                    bc_t = pa.tile([32, TA], F16, tag="bct", name="bct", bufs=1)
                    nc.scalar.copy(bc_t[:], dbl_ps[DT_RANK:64, :])
                    nc.sync.dma_start(bc_d[t][:, t0:t0 + TA], bc_t[:])
                    if (lt + 1) * TA % TB == 0:
                        dtprep_lt(t, ((lt + 1) * TA) // TB - 1)

            # ========== PHASE B: selective scan ==========
            def phaseB(t, after_lt=None):
                tb = TBI[t]
                carries = [None] * 4
                for lt in range(nlt_b):
                    t0 = lt * TB
                    # per-lt inputs shared across n: dt, xs, dxt = dt*xs
                    dtt_t, xst_t, dxt_t = [], [], []
                    for dti in range(4):
                        dtt = pb.tile([128, TB], F16, tag=f"dtt{dti}", name=f"dtt{dti}", bufs=1)
                        nc.gpsimd.dma_start(dtt[:], dt_d[t][dti * 128:(dti + 1) * 128, t0:t0 + TB])
                        xst = pb.tile([128, TB], F16, tag=f"xs{dti}", name=f"xs{dti}", bufs=1)
                        nc.gpsimd.dma_start(xst[:], xs_d[t][dti * 128:(dti + 1) * 128, t0:t0 + TB])
                        dxt = pb.tile([128, TB], F16, tag=f"dtx{dti}", name=f"dtx{dti}", bufs=1)
                        nc.vector.tensor_mul(dxt[:], dtt[:], xst[:])
                        dtt_t.append(dtt)
                        xst_t.append(xst)
                        dxt_t.append(dxt)
                    cnew = [pb.tile([128, D_STATE], F32, tag=f"carry{dti}",
                                    name=f"carry{dti}") for dti in range(4)]
                    for half in range(2):
                        dts = (2 * half, 2 * half + 1)
                        yps = {dti: pb_yps.tile([128, TB], F32, tag=f"yps{i}",
                                                name=f"yps{i}")
                               for i, dti in enumerate(dts)}
                        for n in range(D_STATE):
                            bn = pbc.tile([128, TB], F16, tag="Bn", name="Bn", bufs=3)
                            nc.gpsimd.dma_start(
                                bn[:], bc_d[t][n:n + 1, t0:t0 + TB].partition_broadcast(128))
                            cn = pbc.tile([128, TB], F16, tag="Cn", name="Cn", bufs=3)
                            nc.gpsimd.dma_start(
                                cn[:], bc_d[t][16 + n:17 + n, t0:t0 + TB].partition_broadcast(128))
                            for i, dti in enumerate(dts):
                                a_t = pb.tile([128, TB], F16, tag=f"a{i}", name=f"a{i}", bufs=3)
                                nc.scalar.activation(a_t[:], dtt_t[dti][:], AF.Exp,
                                                     scale=An_sb[t][:, dti * 16 + n:dti * 16 + n + 1])
                                u_t = pb.tile([128, TB], F16, tag=f"u{i}", name=f"u{i}")
                                nc.vector.tensor_mul(u_t[:], dxt_t[dti][:], bn[:])
                                h_t = pb.tile([128, TB], F16, tag=f"h{i}", name=f"h{i}")
                                init = 0.0 if lt == 0 else carries[dti][:, n:n + 1]
                                nc.vector.tensor_tensor_scan(h_t[:], a_t[:], u_t[:], init,
                                                             op0=OP.mult, op1=OP.add)
                                if lt < nlt_b - 1:
                                    nc.scalar.copy(cnew[dti][:, n:n + 1],
                                                   h_t[:, TB - 1:TB])
                                tmp = pb.tile([128, TB], F16, tag=f"tmp{i}", name=f"tmp{i}", bufs=4)
                                nc.vector.tensor_mul(tmp[:], h_t[:], cn[:])
                                for q in range(nqb):
                                    nc.tensor.matmul(yps[dti][:, q * 512:(q + 1) * 512], id_sb[:],
                                                     tmp[:, q * 512:(q + 1) * 512],
                                                     start=(n == 0), stop=False)
                        for dti in dts:
                            # D*x skip term as one diagonal matmul, closes the group
                            w0 = (tb * 4 + dti) * 128
                            for q in range(nqb):
                                nc.tensor.matmul(yps[dti][:, q * 512:(q + 1) * 512],
                                                 dd_sb[:, w0:w0 + 128],
                                                 xst_t[dti][:, q * 512:(q + 1) * 512],
                                                 start=False, stop=True)
                            szt = pb.tile([128, TB], F16, tag="szt", name="szt", bufs=1)
                            if t == "r":
                                nc.gpsimd.dma_start(
                                    szt[:], szl_d["f"][dti * 128:(dti + 1) * 128,
                                                       L - t0 - TB:L - t0])
                                sz_in = rev_ap(szt[:], TB)
                            else:
                                nc.gpsimd.dma_start(szt[:],
                                                    szl_d[t][dti * 128:(dti + 1) * 128, t0:t0 + TB])
                                sz_in = szt[:]
                            ysb = pb.tile([128, TB], F16, tag="ysb", name="ysb", bufs=1)
                            nc.scalar.copy(ysb[:], yps[dti][:])
                            yo = pb.tile([128, TB], F16, tag="yo", name="yo")
                            nc.vector.tensor_mul(yo[:], ysb[:], sz_in)
                            nc.gpsimd.dma_start(y_d[t][dti * 128:(dti + 1) * 128, t0:t0 + TB],
                                                yo[:])
                    carries = cnew
                    if after_lt is not None:
                        after_lt(lt)

            # ========== PHASE C: combine + out_proj ==========
            def phaseC_lt(lt):
                t0 = lt * TB
                Y_t = []
                for dti in range(4):
                    yf = pc.tile([128, TB], F16, tag="yf", name="yf", bufs=1)
                    nc.gpsimd.dma_start(yf[:], y_d["f"][dti * 128:(dti + 1) * 128, t0:t0 + TB])
                    yg = pc.tile([128, TB], F16, tag="yg", name="yg", bufs=1)
                    nc.gpsimd.dma_start(yg[:], y_d["g"][dti * 128:(dti + 1) * 128, t0:t0 + TB])
                    yrf = pc.tile([128, TB], F16, tag="yrf", name="yrf", bufs=1)
                    nc.gpsimd.dma_start(
                        yrf[:], y_d["r"][dti * 128:(dti + 1) * 128, L - t0 - TB:L - t0])
                    yr = pc.tile([128, TB], F16, tag=f"Y{dti}", name=f"Y{dti}", bufs=1)
                    sg = pc.tile([128, TB], F16, tag="sg", name="sg", bufs=1)
                    nc.scalar.activation(sg[:], yg[:], AF.Silu)
                    nc.vector.tensor_add(yr[:], yf[:], rev_ap(yrf[:], TB))
                    nc.vector.tensor_mul(yr[:], yr[:], sg[:])
                    Y_t.append(yr)
                for mt in range(4):
                    for q in range(nqb):
                        ops = pa_ps.tile([128, 512], F32, tag="ps", name="ps")
                        for dti in range(4):
                            nc.tensor.matmul(ops[:], op_sb[dti][:, mt * 128:(mt + 1) * 128],
                                             Y_t[dti][:, q * 512:(q + 1) * 512],
                                             start=(dti == 0), stop=(dti == 3))
                        ot = pc.tile([128, 512], F32, tag="ot", name="ot", bufs=1)
                        nc.scalar.copy(ot[:], ops[:])
                        nc.sync.dma_start(
                            outp[mt * 128:(mt + 1) * 128, t0 + q * 512:t0 + (q + 1) * 512],
                            ot[:])

            # ---- schedule: overlap A(r)/A(g) under B(f)/B(r) ----
            phaseA("f")
            phaseA("r")
            phaseB("f")
            phaseA("g")
            phaseB("r")
            phaseB("g", after_lt=phaseC_lt)

    nc.finalize()
    return nc


def prep_core_inputs(inputs, c, L):
    """Build the input dict for core c (b = c//2, dh = c%2).

    Channels of d_inner are permuted per core so the local half is always
    first: perm = [dh*512 .. dh*512+511, other half].
    """
    b, dh = divmod(c, 2)
    f16 = np.float16
    f32 = np.float32
    perm = np.concatenate([np.arange(dh * 512, dh * 512 + 512),
                           np.arange((1 - dh) * 512, (1 - dh) * 512 + 512)])
    loc = perm[:512]

    hid = np.asarray(inputs["hidden_states"][b], dtype=f32)[:L]
    ano = np.asarray(inputs["another_hidden_states"][b], dtype=f32)[:L]
    d = {
        "inp_f": np.ascontiguousarray(hid.T).astype(f16),
        "inp_g": np.ascontiguousarray(ano.T).astype(f16),
        "Wx": np.ascontiguousarray(inputs["in_proj_w"][:D_INNER][perm].T).astype(f16),
        "Wz": np.ascontiguousarray(inputs["in_proj_w"][D_INNER:][loc].T).astype(f16),
        "Gx": np.ascontiguousarray(inputs["in_proj_g_w"][:D_INNER][perm].T).astype(f16),
        "Gz": np.ascontiguousarray(inputs["in_proj_g_w"][D_INNER:][loc].T).astype(f16),
        "opw": np.ascontiguousarray(inputs["out_proj_w"][:, loc].T).astype(f16),
        "ident": np.eye(128, dtype=f16),
    }
    cwdiag = np.zeros((128, 96 * 128), dtype=f16)
    ddiag = np.zeros((128, 12 * 128), dtype=f16)
    di = np.arange(128)
    for tb, (t, cwn, cbn, xpn, dtwn, dtbn, alogn, dn) in enumerate((
            ("f", "convw_f", "convb_f", "xproj_f", "dtw_f", "dtb_f", "Alog_f", "D_f"),
            ("r", "convw_r", "convb_r", "xproj_r", "dtw_r", "dtb_r", "Alog_r", "D_r"),
            ("g", "convw_g", "convb_g", "xproj_g", "dtw_g", "dtb_g", "Alog_g", "D_g"))):
        cwp = np.asarray(inputs[cwn], f32)[perm]          # (1024, 4)
        for d8 in range(8):
            for k in range(4):
                w0 = (tb * 32 + d8 * 4 + k) * 128
                cwdiag[di, w0 + di] = cwp[d8 * 128:(d8 + 1) * 128, k].astype(f16)
        cbp = np.asarray(inputs[cbn], f32)[perm]          # (1024,)
        d[f"cb_{t}"] = np.ascontiguousarray(cbp.reshape(8, 128).T).astype(f32)
        d[f"xp_{t}"] = np.ascontiguousarray(np.asarray(inputs[xpn], f32).T[perm]).astype(f16)
        d[f"dtw_{t}"] = np.ascontiguousarray(np.asarray(inputs[dtwn], f32)[loc].T).astype(f32)
        dtbp = np.asarray(inputs[dtbn], f32)[loc]
        d[f"dtb_{t}"] = np.ascontiguousarray(dtbp.reshape(4, 128).T).astype(f32)
        Afull = -np.exp(np.asarray(inputs[alogn], f32))[loc]   # (512, 16)
        d[f"An_{t}"] = np.ascontiguousarray(
            Afull.reshape(4, 128, 16).transpose(1, 0, 2).reshape(128, 64)).astype(f32)
        Dp = np.asarray(inputs[dn], f32)[loc]
        for dti in range(4):
            w0 = (tb * 4 + dti) * 128
            ddiag[di, w0 + di] = Dp[dti * 128:(dti + 1) * 128].astype(f16)
    d["cwdiag"] = cwdiag
    d["ddiag"] = ddiag
    return d


_NC_CACHE = {}
TRACE = False
LAST_RESULT = None
BUILD_KW = {}


def kernel(**inputs):
    global LAST_RESULT
    L = inputs["hidden_states"].shape[1]
    key = (L, tuple(sorted(BUILD_KW.items())))
    if key not in _NC_CACHE:
        _NC_CACHE[key] = build(L, **BUILD_KW)
    nc = _NC_CACHE[key]
    in_maps = [prep_core_inputs(inputs, c, L) for c in range(NCORE)]
    res = run_bass_kernel_spmd(nc, in_maps, core_ids=list(range(NCORE)),
                               trace=TRACE)
    LAST_RESULT = res
    outs = []
    for b in range(NB):
        p = res.results[2 * b]["outp"].astype(np.float32) + \
            res.results[2 * b + 1]["outp"].astype(np.float32)
        outs.append(p.T)
    return np.stack(outs).astype(np.float32)


if __name__ == "__main__":
    nc = build(512)
    print("built ok")


# revision 15
# speedup vs baseline: 1.1426x; 1.0727x over previous
"""CDMamba Trainium2 kernel.

Sharding: 8 cores = 4 batches x 2 halves of d_inner (512 channels each).
Each core computes the full x-path (in_proj x-part, conv, silu, x_proj) for
all 1024 channels (duplicated across the pair, so the d_inner contraction in
x_proj needs no collective); the z-path, dt, selective scan, gating and
out_proj run on the local 512 channels only.  out_proj partials (contraction
over d_inner) are summed on the host.  The per-core channel order is
permuted on the host so the local half is always channels 0..511 — the
program is SPMD-identical, only the data differs.

On-chip layout is d-major: [d partitions, time free].  The selective scan
uses the DVE tensor_tensor_scan instruction (state = a_t*state + u_t per
partition row), one scan per (d-tile, state index n).  The n-contraction
y = sum_n C_n * h_n accumulates in PSUM via identity matmuls on the tensor
engine; the D*x skip term is folded in as one extra diagonal matmul.

The DVE (vector engine) is the bottleneck: the scan runs at 2 cycles per
free element and can ONLY run there (gpsimd shares its SBUF port with the
DVE, so offloading the elementwise muls to gpsimd slows the scans down and
loses).  Therefore the schedule maximises DVE occupancy: phases are emitted
in the order A(f) A(r) B(f) A(g) B(r) B(g)+C so the tensor/scalar-bound
phase A of branches r and g overlaps the DVE-bound phase B of earlier
branches (per-engine queues are in emission order; A(x) is emitted *before*
the B slice it overlaps so its tensor work is not stuck behind B's
PSUM-accumulation matmuls).  To make the working sets coexist in SBUF,
dt round-trips through DRAM and the B/C broadcast tiles are loaded n-outer
with rotating buffers (the 2x4 dti split keeps PSUM at exactly 8 banks).
Softplus Exp/Ln activations are batched per phase to avoid reloading the
scalar-engine activation table (1.28us per function switch).
"""

import sys

import numpy as np

for _p in ("/opt/trn_rl_repo",):
    if _p not in sys.path:
        sys.path.insert(0, _p)

import concourse.bass as bass
import concourse.bacc as bacc
import concourse.tile as tile
from concourse import library_config
from concourse import mybir
from concourse.bass_utils import run_bass_kernel_spmd

F16 = mybir.dt.float16
F32 = mybir.dt.float32
AF = mybir.ActivationFunctionType
OP = mybir.AluOpType

D_MODEL = 512
D_STATE = 16
D_CONV = 4
D_INNER = 1024
DT_RANK = 32
NB = 4
NCORE = 8
DLOC = 512
L_FULL = 4096

BRANCHES = ("f", "r", "g")
TBI = {"f": 0, "r": 1, "g": 2}


def rev_ap(src, n):
    """Reversed view (along free dim) of the first n columns ending at src's
    last column."""
    return bass.AP(tensor=src.tensor, offset=src.offset + (n - 1),
                   ap=[list(src.ap[0]), [-1, n]])


def build(L, TA=512, TB=1024):
    """Build the SPMD Bass program for sequence length L."""
    TA = min(TA, L)
    TB = min(TB, L)
    assert L % TA == 0 and L % TB == 0
    nlt_a = L // TA
    nlt_b = L // TB
    nqb = TB // 512

    nc = bacc.Bacc()

    # ---- I/O ----
    inp = {t: nc.declare_dram_parameter(f"inp_{t}", [D_MODEL, L], F16, isOutput=False)
           for t in ("f", "g")}
    Wx = nc.declare_dram_parameter("Wx", [D_MODEL, D_INNER], F16, isOutput=False)
    Wz = nc.declare_dram_parameter("Wz", [D_MODEL, DLOC], F16, isOutput=False)
    Gx = nc.declare_dram_parameter("Gx", [D_MODEL, D_INNER], F16, isOutput=False)
    Gz = nc.declare_dram_parameter("Gz", [D_MODEL, DLOC], F16, isOutput=False)
    cwdiag = nc.declare_dram_parameter("cwdiag", [128, 96 * 128], F16, isOutput=False)
    cb = {t: nc.declare_dram_parameter(f"cb_{t}", [128, 8], F32, isOutput=False) for t in BRANCHES}
    xp = {t: nc.declare_dram_parameter(f"xp_{t}", [D_INNER, 64], F16, isOutput=False) for t in BRANCHES}
    dtw = {t: nc.declare_dram_parameter(f"dtw_{t}", [DT_RANK, DLOC], F32, isOutput=False) for t in BRANCHES}
    dtb = {t: nc.declare_dram_parameter(f"dtb_{t}", [128, 4], F32, isOutput=False) for t in BRANCHES}
    An = {t: nc.declare_dram_parameter(f"An_{t}", [128, 64], F32, isOutput=False) for t in BRANCHES}
    ddiag = nc.declare_dram_parameter("ddiag", [128, 12 * 128], F16, isOutput=False)
    opw = nc.declare_dram_parameter("opw", [DLOC, D_MODEL], F16, isOutput=False)
    ident = nc.declare_dram_parameter("ident", [128, 128], F16, isOutput=False)
    outp = nc.declare_dram_parameter("outp", [D_MODEL, L], F32, isOutput=True)

    # ---- DRAM scratch ----
    xz_d = nc.dram_tensor("xz_d", [D_INNER, L], F16)
    xs_d = {t: nc.dram_tensor(f"xs_{t}", [DLOC, L], F16) for t in BRANCHES}
    szl_d = {t: nc.dram_tensor(f"szl_{t}", [DLOC, L], F16) for t in ("f", "g")}
    y_d = {t: nc.dram_tensor(f"y_{t}", [DLOC, L], F16) for t in BRANCHES}
    ddt_d = {t: nc.dram_tensor(f"ddt_{t}", [DT_RANK, L], F32) for t in BRANCHES}
    bc_d = {t: nc.dram_tensor(f"bc_{t}", [32, L], F16) for t in BRANCHES}
    dt_d = {t: nc.dram_tensor(f"dt_{t}", [DLOC, L], F16) for t in BRANCHES}

    with tile.TileContext(nc) as tc:
        with tc.tile_pool(name="wpool", bufs=1) as wp, \
             tc.tile_pool(name="pa_ps", bufs=3, space="PSUM") as pa_ps, \
             tc.tile_pool(name="pa_sm", bufs=1, space="PSUM") as pa_sm, \
             tc.tile_pool(name="pb", bufs=2) as pb, \
             tc.tile_pool(name="pb_yps", bufs=1, space="PSUM") as pb_yps, \
             tc.tile_pool(name="paw", bufs=1) as paw, \
             tc.tile_pool(name="pa", bufs=2) as pa, \
             tc.tile_pool(name="pbc", bufs=1) as pbc, \
             tc.tile_pool(name="pc", bufs=2) as pc:

            # ---- persistent weights in SBUF ----
            def load_w(pool, name, dram, parts, width, dtype=F16):
                tl = []
                for k in range(parts):
                    t_ = pool.tile([128, width], dtype, tag=f"{name}{k}", name=f"{name}{k}")
                    nc.sync.dma_start(t_[:], dram[k * 128:(k + 1) * 128, :])
                    tl.append(t_)
                return tl

            xp_sb, dtw_sb, cb_sb, dtb_sb, An_sb = {}, {}, {}, {}, {}
            for t in BRANCHES:
                xp_sb[t] = wp.tile([128, 8 * 64], F16, tag=f"xp_{t}", name=f"xp_{t}")
                for k in range(8):
                    nc.sync.dma_start(xp_sb[t][:, k * 64:(k + 1) * 64],
                                      xp[t][k * 128:(k + 1) * 128, :])
                dtw_sb[t] = wp.tile([DT_RANK, DLOC], F32, tag=f"dtw_{t}", name=f"dtw_{t}")
                nc.sync.dma_start(dtw_sb[t][:], dtw[t][:])
                for nm, src, shape in (("cb", cb, [128, 8]),
                                       ("dtb", dtb, [128, 4]), ("An", An, [128, 64])):
                    t_ = wp.tile(shape, F32, tag=f"{nm}_{t}")
                    nc.sync.dma_start(t_[:], src[t][:])
                    {"cb": cb_sb, "dtb": dtb_sb, "An": An_sb}[nm][t] = t_
            op_sb = load_w(wp, "op", opw, 4, D_MODEL)
            id_sb = wp.tile([128, 128], F16, tag="ident", name="ident")
            nc.sync.dma_start(id_sb[:], ident[:])
            dd_sb = wp.tile([128, 12 * 128], F16, tag="ddg", name="ddg")
            nc.sync.dma_start(dd_sb[:], ddiag[:])
            zt_sb = wp.tile([128, 4], F16, tag="zt4", name="zt4")
            nc.gpsimd.memset(zt_sb[:], 0.0)

            wx_sb = {"Wx": load_w(paw, "Wx", Wx, 4, D_INNER),
                     "Gx": load_w(paw, "Gx", Gx, 4, D_INNER)}
            wz_sb = {"Wz": load_w(paw, "Wz", Wz, 4, DLOC),
                     "Gz": load_w(paw, "Gz", Gz, 4, DLOC)}
            in_x_sb = {"f": wx_sb["Wx"], "g": wx_sb["Gx"]}
            in_z_sb = {"f": wz_sb["Wz"], "g": wz_sb["Gz"]}

            tc.strict_bb_all_engine_barrier()

            # ========== dt prep: dt = softplus(dtw @ ddt + dtb) -> dt_d ====
            def dtprep_lt(t, lt):
                t0 = lt * TB
                ddt_sb = pb.tile([DT_RANK, TB], F32, tag="ddt_sb", name="ddt_sb",
                                 bufs=1)
                nc.gpsimd.dma_start(ddt_sb[:], ddt_d[t][:, t0:t0 + TB])
                # softplus(x) = ln(exp(x) + 1); x in [-8, 2] here.  All Exp
                # activations run back-to-back, then all Ln — alternating
                # them reloads the scalar act table (1.28us per switch).
                rows = []
                for grp in range(2):
                    spes = []
                    for dti in (2 * grp, 2 * grp + 1):
                        dtt = pb.tile([128, TB], F16, tag=f"dtp{dti}", name=f"dtp{dti}", bufs=1)
                        for q in range(nqb):
                            dps = pa_ps.tile([128, 512], F32, tag="ps", name="ps")
                            nc.tensor.matmul(dps[:],
                                             dtw_sb[t][:, dti * 128:(dti + 1) * 128],
                                             ddt_sb[:, q * 512:(q + 1) * 512],
                                             start=True, stop=True)
                            spe = pb.tile([128, 512], F16, tag=f"spe{dti % 2}_{q}",
                                          name=f"spe{dti % 2}_{q}", bufs=1)
                            nc.scalar.activation(spe[:], dps[:], AF.Exp,
                                                 bias=dtb_sb[t][:, dti:dti + 1])
                            spes.append((dtt, q, spe))
                        rows.append(dtt)
                    for dtt, q, spe in spes:
                        nc.scalar.activation(dtt[:, q * 512:(q + 1) * 512], spe[:],
                                             AF.Ln, bias=1.0)
                for dti in range(4):
                    nc.sync.dma_start(dt_d[t][dti * 128:(dti + 1) * 128, t0:t0 + TB],
                                      rows[dti][:])

            # ========== PHASE A: in_proj + conv(PE) + silu + x_proj ==========
            def phaseA(t):
                tb = TBI[t]
                cwd_sb = paw.tile([128, 32 * 128], F16, tag="cwd", name="cwd",
                                  bufs=1)
                nc.sync.dma_start(cwd_sb[:],
                                  cwdiag[:, tb * 32 * 128:(tb + 1) * 32 * 128])
                xz_prev = [None] * 8
                for lt in range(nlt_a):
                    t0 = lt * TA
                    rhs = []
                    if t != "r":
                        for k in range(4):
                            r_ = pa.tile([128, TA], F16, tag=f"rhs{k}", name=f"rhs{k}", bufs=1)
                            nc.sync.dma_start(r_[:], inp[t][k * 128:(k + 1) * 128, t0:t0 + TA])
                            rhs.append(r_)
                    dbl_ps = pa_sm.tile([64, TA], F32, tag="dbl", name="dbl")
                    for d8 in range(8):
                        if t == "r":
                            # xz_r = time-reversed xz_f: forward DMA, then the
                            # conv matmuls read it through a negative-stride
                            # (reversed) moving AP — no on-chip reversal copy.
                            # Layout: xzf = forward data; window k of the
                            # reversed stream is reversed(xzf[3-k : 3-k+TA]).
                            # At lt==0 columns [TA, TA+3) are the zero history.
                            xzf = pa.tile([128, TA + 3], F16, tag="xzf", name="xzf", bufs=2)
                            if lt == 0:
                                nc.sync.dma_start(
                                    xzf[:, 0:TA],
                                    xz_d[d8 * 128:(d8 + 1) * 128, L - TA:L])
                                nc.gpsimd.memset(xzf[:, TA:TA + 3], 0.0)
                            else:
                                nc.sync.dma_start(
                                    xzf[:, 0:TA + 3],
                                    xz_d[d8 * 128:(d8 + 1) * 128, L - t0 - TA:L - t0 + 3])
                            conv_in = [rev_ap(xzf[:, 3 - k:3 - k + TA], TA)
                                       for k in range(4)]
                        else:
                            xz = pa.tile([128, TA + 3], F16, tag=f"xz{d8}", name=f"xz{d8}")
                            ps = pa_ps.tile([128, TA], F32, tag="ps", name="ps")
                            for k in range(4):
                                nc.tensor.matmul(ps[:], in_x_sb[t][k][:, d8 * 128:(d8 + 1) * 128],
                                                 rhs[k][:], start=(k == 0), stop=(k == 3))
                            if lt == 0:
                                nc.scalar.copy(xz[:, 0:3], zt_sb[:, 0:3])
                            else:
                                nc.scalar.copy(xz[:, 0:3], xz_prev[d8][:, TA:TA + 3])
                            nc.scalar.copy(xz[:, 3:TA + 3], ps[:])
                            xz_prev[d8] = xz
                            if t == "f":
                                nc.sync.dma_start(xz_d[d8 * 128:(d8 + 1) * 128, t0:t0 + TA],
                                                  xz[:, 3:TA + 3])
                            conv_in = [xz[:, k:k + TA] for k in range(4)]
                        # conv: 4 shifted diagonal matmuls accumulating in PSUM
                        cps = pa_ps.tile([128, TA], F32, tag="ps", name="ps")
                        for k in range(4):
                            w0 = (d8 * 4 + k) * 128
                            nc.tensor.matmul(cps[:], cwd_sb[:, w0:w0 + 128],
                                             conv_in[k], start=(k == 0), stop=(k == 3))
                        xs = pa.tile([128, TA], F16, tag="xs", name="xs")
                        nc.scalar.activation(xs[:], cps[:], AF.Silu,
                                             bias=cb_sb[t][:, d8:d8 + 1])
                        nc.tensor.matmul(dbl_ps[:], xp_sb[t][:, d8 * 64:(d8 + 1) * 64],
                                         xs[:], start=(d8 == 0), stop=(d8 == 7))
                        if d8 < 4:  # local half (host permutes channels)
                            nc.sync.dma_start(xs_d[t][d8 * 128:(d8 + 1) * 128, t0:t0 + TA],
                                              xs[:])
                    if t != "r":
                        for zt in range(4):
                            zps = pa_ps.tile([128, TA], F32, tag="ps", name="ps")
                            for k in range(4):
                                nc.tensor.matmul(zps[:], in_z_sb[t][k][:, zt * 128:(zt + 1) * 128],
                                                 rhs[k][:], start=(k == 0), stop=(k == 3))
                            sz = pa.tile([128, TA], F16, tag="sz", name="sz")
                            nc.scalar.activation(sz[:], zps[:], AF.Silu)
                            nc.sync.dma_start(szl_d[t][zt * 128:(zt + 1) * 128, t0:t0 + TA], sz[:])
                    ddt_t = pa.tile([DT_RANK, TA], F32, tag="ddt", name="ddt", bufs=1)
                    nc.scalar.copy(ddt_t[:], dbl_ps[0:DT_RANK, :])
                    nc.sync.dma_start(ddt_d[t][:, t0:t0 + TA], ddt_t[:])
                    bc_t = pa.tile([32, TA], F16, tag="bct", name="bct", bufs=1)
                    nc.scalar.copy(bc_t[:], dbl_ps[DT_RANK:64, :])
                    nc.sync.dma_start(bc_d[t][:, t0:t0 + TA], bc_t[:])
                    if (lt + 1) * TA % TB == 0:
                        dtprep_lt(t, ((lt + 1) * TA) // TB - 1)

            # ========== PHASE B: selective scan ==========
            def phaseB(t, after_lt=None):
                tb = TBI[t]
                carries = [None] * 4
                for lt in range(nlt_b):
                    t0 = lt * TB
                    # per-lt inputs shared across n: dt, xs, dxt = dt*xs
                    dtt_t, xst_t, dxt_t = [], [], []
                    for dti in range(4):
                        dtt = pb.tile([128, TB], F16, tag=f"dtt{dti}", name=f"dtt{dti}", bufs=1)
                        nc.gpsimd.dma_start(dtt[:], dt_d[t][dti * 128:(dti + 1) * 128, t0:t0 + TB])
                        xst = pb.tile([128, TB], F16, tag=f"xs{dti}", name=f"xs{dti}", bufs=1)
                        nc.gpsimd.dma_start(xst[:], xs_d[t][dti * 128:(dti + 1) * 128, t0:t0 + TB])
                        dxt = pb.tile([128, TB], F16, tag=f"dtx{dti}", name=f"dtx{dti}", bufs=1)
                        nc.vector.tensor_mul(dxt[:], dtt[:], xst[:])
                        dtt_t.append(dtt)
                        xst_t.append(xst)
                        dxt_t.append(dxt)
                    cnew = [pb.tile([128, D_STATE], F32, tag=f"carry{dti}",
                                    name=f"carry{dti}") for dti in range(4)]
                    for half in range(2):
                        dts = (2 * half, 2 * half + 1)
                        yps = {dti: pb_yps.tile([128, TB], F32, tag=f"yps{i}",
                                                name=f"yps{i}")
                               for i, dti in enumerate(dts)}
                        for n in range(D_STATE):
                            bn = pbc.tile([128, TB], F16, tag="Bn", name="Bn", bufs=3)
                            nc.gpsimd.dma_start(
                                bn[:], bc_d[t][n:n + 1, t0:t0 + TB].partition_broadcast(128))
                            cn = pbc.tile([128, TB], F16, tag="Cn", name="Cn", bufs=3)
                            nc.gpsimd.dma_start(
                                cn[:], bc_d[t][16 + n:17 + n, t0:t0 + TB].partition_broadcast(128))
                            for i, dti in enumerate(dts):
                                a_t = pb.tile([128, TB], F16, tag=f"a{i}", name=f"a{i}", bufs=4)
                                nc.scalar.activation(a_t[:], dtt_t[dti][:], AF.Exp,
                                                     scale=An_sb[t][:, dti * 16 + n:dti * 16 + n + 1])
                                u_t = pb.tile([128, TB], F16, tag=f"u{i}", name=f"u{i}")
                                nc.vector.tensor_mul(u_t[:], dxt_t[dti][:], bn[:])
                                h_t = pb.tile([128, TB], F16, tag=f"h{i}", name=f"h{i}")
                                init = 0.0 if lt == 0 else carries[dti][:, n:n + 1]
                                nc.vector.tensor_tensor_scan(h_t[:], a_t[:], u_t[:], init,
                                                             op0=OP.mult, op1=OP.add)
                                if lt < nlt_b - 1:
                                    nc.scalar.copy(cnew[dti][:, n:n + 1],
                                                   h_t[:, TB - 1:TB])
                                tmp = pb.tile([128, TB], F16, tag=f"tmp{i}", name=f"tmp{i}", bufs=4)
                                nc.vector.tensor_mul(tmp[:], h_t[:], cn[:])
                                for q in range(nqb):
                                    nc.tensor.matmul(yps[dti][:, q * 512:(q + 1) * 512], id_sb[:],
                                                     tmp[:, q * 512:(q + 1) * 512],
                                                     start=(n == 0), stop=False)
                        for dti in dts:
                            # D*x skip term as one diagonal matmul, closes the group
                            w0 = (tb * 4 + dti) * 128
                            for q in range(nqb):
                                nc.tensor.matmul(yps[dti][:, q * 512:(q + 1) * 512],
                                                 dd_sb[:, w0:w0 + 128],
                                                 xst_t[dti][:, q * 512:(q + 1) * 512],
                                                 start=False, stop=True)
                            szt = pb.tile([128, TB], F16, tag="szt", name="szt", bufs=1)
                            if t == "r":
                                nc.gpsimd.dma_start(
                                    szt[:], szl_d["f"][dti * 128:(dti + 1) * 128,
                                                       L - t0 - TB:L - t0])
                                sz_in = rev_ap(szt[:], TB)
                            else:
                                nc.gpsimd.dma_start(szt[:],
                                                    szl_d[t][dti * 128:(dti + 1) * 128, t0:t0 + TB])
                                sz_in = szt[:]
                            ysb = pb.tile([128, TB], F16, tag="ysb", name="ysb", bufs=1)
                            nc.scalar.copy(ysb[:], yps[dti][:])
                            yo = pb.tile([128, TB], F16, tag="yo", name="yo")
                            nc.vector.tensor_mul(yo[:], ysb[:], sz_in)
                            nc.gpsimd.dma_start(y_d[t][dti * 128:(dti + 1) * 128, t0:t0 + TB],
                                                yo[:])
                    carries = cnew
                    if after_lt is not None:
                        after_lt(lt)

            # ========== PHASE C: combine + out_proj ==========
            def phaseC_lt(lt):
                t0 = lt * TB
                Y_t = []
                for dti in range(4):
                    yf = pc.tile([128, TB], F16, tag="yf", name="yf", bufs=1)
                    nc.gpsimd.dma_start(yf[:], y_d["f"][dti * 128:(dti + 1) * 128, t0:t0 + TB])
                    yg = pc.tile([128, TB], F16, tag="yg", name="yg", bufs=1)
                    nc.gpsimd.dma_start(yg[:], y_d["g"][dti * 128:(dti + 1) * 128, t0:t0 + TB])
                    yrf = pc.tile([128, TB], F16, tag="yrf", name="yrf", bufs=1)
                    nc.gpsimd.dma_start(
                        yrf[:], y_d["r"][dti * 128:(dti + 1) * 128, L - t0 - TB:L - t0])
                    yr = pc.tile([128, TB], F16, tag=f"Y{dti}", name=f"Y{dti}", bufs=1)
                    sg = pc.tile([128, TB], F16, tag="sg", name="sg", bufs=1)
                    nc.scalar.activation(sg[:], yg[:], AF.Silu)
                    nc.vector.tensor_add(yr[:], yf[:], rev_ap(yrf[:], TB))
                    nc.vector.tensor_mul(yr[:], yr[:], sg[:])
                    Y_t.append(yr)
                for mt in range(4):
                    for q in range(nqb):
                        ops = pa_ps.tile([128, 512], F32, tag="ps", name="ps")
                        for dti in range(4):
                            nc.tensor.matmul(ops[:], op_sb[dti][:, mt * 128:(mt + 1) * 128],
                                             Y_t[dti][:, q * 512:(q + 1) * 512],
                                             start=(dti == 0), stop=(dti == 3))
                        ot = pc.tile([128, 512], F32, tag="ot", name="ot", bufs=1)
                        nc.scalar.copy(ot[:], ops[:])
                        nc.sync.dma_start(
                            outp[mt * 128:(mt + 1) * 128, t0 + q * 512:t0 + (q + 1) * 512],
                            ot[:])

            # ---- schedule: overlap A(r)/A(g) under B(f)/B(r) ----
            phaseA("f")
            phaseB("f")
            phaseA("r")
            phaseB("r")
            phaseA("g")
            phaseB("g", after_lt=phaseC_lt)

    nc.finalize()
    return nc


def prep_core_inputs(inputs, c, L):
    """Build the input dict for core c (b = c//2, dh = c%2).

    Channels of d_inner are permuted per core so the local half is always
    first: perm = [dh*512 .. dh*512+511, other half].
    """
    b, dh = divmod(c, 2)
    f16 = np.float16
    f32 = np.float32
    perm = np.concatenate([np.arange(dh * 512, dh * 512 + 512),
                           np.arange((1 - dh) * 512, (1 - dh) * 512 + 512)])
    loc = perm[:512]

    hid = np.asarray(inputs["hidden_states"][b], dtype=f32)[:L]
    ano = np.asarray(inputs["another_hidden_states"][b], dtype=f32)[:L]
    d = {
        "inp_f": np.ascontiguousarray(hid.T).astype(f16),
        "inp_g": np.ascontiguousarray(ano.T).astype(f16),
        "Wx": np.ascontiguousarray(inputs["in_proj_w"][:D_INNER][perm].T).astype(f16),
        "Wz": np.ascontiguousarray(inputs["in_proj_w"][D_INNER:][loc].T).astype(f16),
        "Gx": np.ascontiguousarray(inputs["in_proj_g_w"][:D_INNER][perm].T).astype(f16),
        "Gz": np.ascontiguousarray(inputs["in_proj_g_w"][D_INNER:][loc].T).astype(f16),
        "opw": np.ascontiguousarray(inputs["out_proj_w"][:, loc].T).astype(f16),
        "ident": np.eye(128, dtype=f16),
    }
    cwdiag = np.zeros((128, 96 * 128), dtype=f16)
    ddiag = np.zeros((128, 12 * 128), dtype=f16)
    di = np.arange(128)
    for tb, (t, cwn, cbn, xpn, dtwn, dtbn, alogn, dn) in enumerate((
            ("f", "convw_f", "convb_f", "xproj_f", "dtw_f", "dtb_f", "Alog_f", "D_f"),
            ("r", "convw_r", "convb_r", "xproj_r", "dtw_r", "dtb_r", "Alog_r", "D_r"),
            ("g", "convw_g", "convb_g", "xproj_g", "dtw_g", "dtb_g", "Alog_g", "D_g"))):
        cwp = np.asarray(inputs[cwn], f32)[perm]          # (1024, 4)
        for d8 in range(8):
            for k in range(4):
                w0 = (tb * 32 + d8 * 4 + k) * 128
                cwdiag[di, w0 + di] = cwp[d8 * 128:(d8 + 1) * 128, k].astype(f16)
        cbp = np.asarray(inputs[cbn], f32)[perm]          # (1024,)
        d[f"cb_{t}"] = np.ascontiguousarray(cbp.reshape(8, 128).T).astype(f32)
        d[f"xp_{t}"] = np.ascontiguousarray(np.asarray(inputs[xpn], f32).T[perm]).astype(f16)
        d[f"dtw_{t}"] = np.ascontiguousarray(np.asarray(inputs[dtwn], f32)[loc].T).astype(f32)
        dtbp = np.asarray(inputs[dtbn], f32)[loc]
        d[f"dtb_{t}"] = np.ascontiguousarray(dtbp.reshape(4, 128).T).astype(f32)
        Afull = -np.exp(np.asarray(inputs[alogn], f32))[loc]   # (512, 16)
        d[f"An_{t}"] = np.ascontiguousarray(
            Afull.reshape(4, 128, 16).transpose(1, 0, 2).reshape(128, 64)).astype(f32)
        Dp = np.asarray(inputs[dn], f32)[loc]
        for dti in range(4):
            w0 = (tb * 4 + dti) * 128
            ddiag[di, w0 + di] = Dp[dti * 128:(dti + 1) * 128].astype(f16)
    d["cwdiag"] = cwdiag
    d["ddiag"] = ddiag
    return d


_NC_CACHE = {}
TRACE = False
LAST_RESULT = None
BUILD_KW = {}


def kernel(**inputs):
    global LAST_RESULT
    L = inputs["hidden_states"].shape[1]
    key = (L, tuple(sorted(BUILD_KW.items())))
    if key not in _NC_CACHE:
        _NC_CACHE[key] = build(L, **BUILD_KW)
    nc = _NC_CACHE[key]
    in_maps = [prep_core_inputs(inputs, c, L) for c in range(NCORE)]
    res = run_bass_kernel_spmd(nc, in_maps, core_ids=list(range(NCORE)),
                               trace=TRACE)
    LAST_RESULT = res
    outs = []
    for b in range(NB):
        p = res.results[2 * b]["outp"].astype(np.float32) + \
            res.results[2 * b + 1]["outp"].astype(np.float32)
        outs.append(p.T)
    return np.stack(outs).astype(np.float32)


if __name__ == "__main__":
    nc = build(512)
    print("built ok")
